# revision 11
# baseline (speedup 1.0000x reference)
"""GAT (2-layer, 8-head) Bass kernel for 8 Trainium2 NeuronCores.

Strategy (edge-parallel, dst-sharded):
  - Nodes are assigned to 8 cores x 49 windows of 128 slots by a
    degree-balanced permutation (computed on host), so every window carries
    ~equal edge count and the shared SPMD tile schedule has minimal padding.
  - Core c owns its 49 windows (6272 node slots) and processes all edges
    whose dst lands there.  Per 128-edge tile a one-hot (edge x window-row)
    matrix is built and a PE matmul accumulates weighted messages into a
    PSUM window; per-edge softmax weight w = exp(leaky_relu(as[src]+ad[dst]))
    with as gathered with the src record and ad fetched via a matmul against
    the transposed one-hot.  Divide + bias + relu at node level; repeat for
    layer 2; output projection.
  - Records are float16, 256 wide (h 128 | as 8 | ad 8 | pad): the
    dma_gather moves 512B/edge and the AllGather of the replicated node
    table moves 3.2MB/core/layer.
  - Work is spread across engines: PE (matmuls, one-hot transposes),
    DVE (one-hot build, message scaling), Act (PSUM->SBUF batch copies,
    exp), Pool/GPSIMD (gathers, leaky-relu), so the edge phase overlaps.

  Transfer format (the axon tunnel dominates end-to-end wall time, so all
  per-core inputs are packed into ONE int16 tensor, ~1.15 MB/core):
  - x ships as int8, quantized per node (scale = max|x_row|/127, f16 scales
    alongside), pre-transposed into 128x128 lhsT tiles.
  - Weights ship as float16 (W1, W2, W@A_blockdiag fused on host, Wout);
    each core carries 1/8th, reassembled on device by a small AllGather.
  - dma_gather index table ships 16-partition-wrapped; dstoff (edge ->
    destination row-in-window) ships as uint8 (255 = pad).
  - The output ships back as float16.
"""

import sys
import os

for _p in ("/opt/trn_rl_repo", "/root/.axon_site/_ro/trn_rl_repo"):
    if os.path.isdir(_p) and _p not in sys.path:
        sys.path.insert(0, _p)

import numpy as np

NEG_SLOPE = 0.2
WW = 128      # window rows = one 128-node block (partition-aligned)


def full_cfg():
    return dict(cores=8, n=50000, tb=49, cb=8, in_ch=128, hc=128,
                heads=8, hid=16, ncls=10)


def derive(cfg):
    d = dict(cfg)
    d["slice_pad"] = d["tb"] * 128              # node slots per core
    d["table_rows"] = d["cores"] * d["slice_pad"]
    d["half_rows"] = d["table_rows"] // 2
    d["trw"] = 256                     # table row width (f16)
    d["mw"] = d["hc"] + d["heads"]     # message width: h|w
    d["chunk"] = 128 * d["cb"]
    d["nwin"] = d["tb"]
    return d


def pack_layout(c, ntot):
    """Row offsets into the per-core [R, 128] int16 pack tensor."""
    K = -(-(ntot * 8) // 128)      # columns (of 128 i16) for idx tables
    Q = -(-ntot // 256)            # 128-row groups for the u8 dstoff table
    ngx = -(-c["tb"] // 2)         # int8 xT tile pairs
    off = {}
    r = 0
    off["xT8"] = r; r += ngx * 128   # i8: tile 2g in bytes 0:128, 2g+1 in 128:256
    off["gidx"] = r; r += 16 * K
    off["dstoff"] = r; r += Q * 128  # u8 [128, ntot] (255 = pad slot)
    # weights are identical on every core: each shard carries 1/8th of the
    # 392-row block (W1 | W2 | misc | rows4 | pad); an on-device AllGather
    # reassembles it. Full-block layout: 0:128 W1, 128:256 W2, 256:384 misc
    # (f16 cols 0:16 WA1 | 16:32 WA2 | 32:42 Wout | 42:42+tb x-scales),
    # 384:388 rows4 (iota | b1 | b2 | bout).
    off["wsh"] = r; r += 392 // 8
    off["R"] = r
    off["K"] = K
    off["Q"] = Q
    return off


# ---------------------------------------------------------------- host prep

def balance_nodes(edge_index, c):
    """Degree-balanced node -> (core, window, pos) assignment.

    Returns (node_core, node_win, node_pos): per-node core id, window id
    within the core (0..tb-1), and row within the window (0..127).
    """
    import heapq
    n, cores, nwin = c["n"], c["cores"], c["nwin"]
    # self-loops are handled node-locally on device, so balance on indegree
    deg = np.bincount(np.asarray(edge_index[1], np.int64) % n,
                      minlength=n).astype(np.int64)
    nw = cores * nwin
    order = np.argsort(-deg, kind="stable")
    heap = [(0, w) for w in range(nw)]
    heapq.heapify(heap)
    count = np.zeros(nw, np.int32)
    node_w = np.zeros(n, np.int32)
    node_pos = np.zeros(n, np.int32)
    for nid in order:
        while True:
            load, w = heapq.heappop(heap)
            if count[w] < WW:
                break
        node_w[nid] = w
        node_pos[nid] = count[w]
        count[w] += 1
        if count[w] < WW:
            heapq.heappush(heap, (load + int(deg[nid]), w))
    return node_w // nwin, node_w % nwin, node_pos


def host_prep(x, edge_index, c):
    """Node permutation + per-core edge maps + shared window schedule."""
    n, cores = c["n"], c["cores"]
    tb, cb, nwin = c["tb"], c["cb"], c["nwin"]
    sp = c["slice_pad"]

    node_core, node_win, node_pos = balance_nodes(edge_index, c)
    # table row of each node (within the full 8-core table)
    trow_n = node_core * sp + node_pos * tb + node_win

    # self-loops (PyG GATConv default) are handled node-locally on device;
    # only the real edges go through the gather/one-hot machinery.
    src = np.asarray(edge_index[0], np.int64)
    dst = np.asarray(edge_index[1], np.int64)
    trow = trow_n[src]
    half = (trow >= c["half_rows"]).astype(np.int64)
    owner = node_core[dst]
    win = node_win[dst]
    pos = node_pos[dst]

    # counts per (core, half, window) in one bincount
    key = (owner * 2 + half) * nwin + win
    counts = np.bincount(key, minlength=cores * 2 * nwin).reshape(cores, 2, nwin)
    tpw = -(-counts.max(axis=0) // 128)          # [2, nwin] tiles per window
    ntiles = tpw.sum(axis=1)
    for h in (0, 1):
        padt = (-int(ntiles[h])) % cb
        if padt:
            nz = np.nonzero(tpw[h])[0]
            wlast = int(nz[-1]) if len(nz) else 0
            tpw[h, wlast] += padt
            ntiles[h] += padt
    sched = dict(tpw=tpw, ntiles=[int(ntiles[0]), int(ntiles[1])])

    ntot = int(ntiles.sum())
    cap = ntot * 128
    # tile start (global, across both halves) for each (half, window)
    tstart = np.zeros((2, nwin), np.int64)
    tstart[0] = np.cumsum(tpw[0]) - tpw[0]
    tstart[1] = int(ntiles[0]) + np.cumsum(tpw[1]) - tpw[1]

    maps = []
    for core in range(cores):
        m = owner == core
        tr_c = trow[m]
        wn_c = win[m]
        ps_c = pos[m]
        hf_c = half[m]
        order = np.lexsort((wn_c, hf_c))
        tr_c, wn_c, ps_c, hf_c = tr_c[order], wn_c[order], ps_c[order], hf_c[order]

        bkey = hf_c * nwin + wn_c
        bcnt = np.bincount(bkey, minlength=2 * nwin)
        bstart = np.cumsum(bcnt) - bcnt             # start idx in sorted order
        rank = np.arange(len(bkey)) - bstart[bkey]
        base = (tstart.reshape(-1)[bkey]) * 128
        slots = base + rank
        assert np.all(rank < tpw.reshape(-1)[bkey] * 128)

        srcrow = np.zeros(cap, np.int64)            # pads: row 0
        dstoff = np.full(cap, 255, np.uint8)        # pads: no one-hot match
        srcrow[slots] = tr_c - hf_c * c["half_rows"]
        dstoff[slots] = ps_c.astype(np.uint8)

        def wrap16(vals):
            nq = ntot // cb
            v = vals.reshape(nq, cb * 128)
            w16 = np.zeros((nq, 16, cb * 8), np.int16)
            k = np.arange(cb * 128)
            w16[:, k % 16, k // 16] = v
            return w16.transpose(1, 0, 2).reshape(16, nq * cb * 8)

        maps.append(dict(
            gidx=wrap16(srcrow.astype(np.int16)),
            dstoff=np.ascontiguousarray(dstoff.reshape(ntot, 128).T),
        ))
    perm = dict(core=node_core, win=node_win, pos=node_pos)
    return maps, sched, perm


def host_pack(x, edge_maps, sched, perm, W1, a_src1, a_dst1, b1, W2, a_src2,
              a_dst2, b2, Wout, bout, c):
    """Assemble the per-core [R, 128] int16 pack tensors."""
    heads, hid, hc, tb = c["heads"], c["hid"], c["hc"], c["tb"]
    sp = c["slice_pad"]
    ntot = int(sched["ntiles"][0] + sched["ntiles"][1])
    lay = pack_layout(c, ntot)

    def blockdiag(a):
        A = np.zeros((hc, heads), np.float32)
        for h in range(heads):
            A[h * hid:(h + 1) * hid, h] = a[h]
        return A

    W1 = np.asarray(W1, np.float32)
    W2 = np.asarray(W2, np.float32)
    WA1 = np.concatenate([W1 @ blockdiag(np.asarray(a_src1, np.float32)),
                          W1 @ blockdiag(np.asarray(a_dst1, np.float32))], axis=1)
    WA2 = np.concatenate([W2 @ blockdiag(np.asarray(a_src2, np.float32)),
                          W2 @ blockdiag(np.asarray(a_dst2, np.float32))], axis=1)
    misc = np.zeros((128, 128), np.float16)
    misc[:, 0:16] = WA1.astype(np.float16)
    misc[:, 16:32] = WA2.astype(np.float16)
    misc[:, 32:42] = np.asarray(Wout, np.float32).astype(np.float16)
    rows4 = np.zeros((4, 128), np.float16)
    rows4[0] = np.arange(128, dtype=np.float16)
    rows4[1] = np.asarray(b1, np.float32).astype(np.float16)
    rows4[2] = np.asarray(b2, np.float32).astype(np.float16)
    rows4[3, 0:c["ncls"]] = np.asarray(bout, np.float32).astype(np.float16)
    wblock = np.zeros((392, 128), np.int16)
    wblock[0:128] = W1.astype(np.float16).view(np.int16)
    wblock[128:256] = W2.astype(np.float16).view(np.int16)
    wblock[256:384] = misc.view(np.int16)
    wblock[384:388] = rows4.view(np.int16)

    x = np.asarray(x, np.float32)
    n = c["n"]
    slot_n = perm["win"].astype(np.int64) * WW + perm["pos"]   # slot in core
    ngx = -(-tb // 2)
    packs = []
    for core in range(c["cores"]):
        em = edge_maps[core]
        p = np.zeros((lay["R"], 128), np.int16)
        xs = np.zeros((sp, c["in_ch"]), np.float32)
        mask = perm["core"] == core
        xs[slot_n[mask]] = x[mask]
        s = np.maximum(np.abs(xs).max(axis=1) / 127.0, 1e-4).astype(np.float16)
        xq = np.round(xs / s[:, None].astype(np.float32)).clip(-127, 127)
        # per-tile transposed lhsT blocks [128 in_ch, 128 nodes], int8
        xT = np.ascontiguousarray(
            xq.astype(np.int8).reshape(tb, 128, c["in_ch"]).transpose(0, 2, 1))
        blocks = np.zeros((ngx, 128, 256), np.int8)
        blocks[:, :, 0:128] = xT[0::2]
        blocks[: tb // 2, :, 128:256] = xT[1::2]
        # per-node dequant scales ride in the unused half of the last
        # (odd-tb) group: [128, tb] f16 as raw bytes
        assert tb % 2 == 1
        blocks[tb // 2, :, 128:128 + 2 * tb] = np.ascontiguousarray(
            s.reshape(tb, 128).T).view(np.int8)
        p[lay["xT8"]:lay["xT8"] + ngx * 128] = \
            blocks.reshape(ngx * 128, 256).view(np.int16)

        K = lay["K"]
        gi = np.zeros((16, K * 128), np.int16)
        gi[:, :ntot * 8] = em["gidx"]
        p[lay["gidx"]:lay["gidx"] + 16 * K] = gi.reshape(16 * K, 128)

        Q = lay["Q"]
        doff = np.full((128, Q * 256), 255, np.uint8)
        doff[:, :ntot] = em["dstoff"]
        p[lay["dstoff"]:lay["dstoff"] + Q * 128] = np.ascontiguousarray(
            doff.reshape(128, Q, 256).transpose(1, 0, 2)
        ).reshape(Q * 128, 256).view(np.int16)

        p[lay["wsh"]:lay["wsh"] + 49] = wblock[core * 49:(core + 1) * 49]
        packs.append(p)
    return packs


def host_post(results, perm, c):
    n, tb = c["n"], c["tb"]
    out = np.zeros((n, c["ncls"]), np.float32)
    rows = perm["pos"].astype(np.int64) * tb + perm["win"]
    for core in range(c["cores"]):
        res = np.asarray(results[core]["out"], np.float32)
        mask = perm["core"] == core
        out[mask] = res[rows[mask]]
    return out


# ---------------------------------------------------------------- device build

def build_nc(c, sched, run_edges=True):
    from concourse import bass, mybir, bacc, tile
    from concourse.masks import make_identity

    f32 = mybir.dt.float32
    f16 = mybir.dt.float16
    i16 = mybir.dt.int16
    i8 = mybir.dt.int8
    u8 = mybir.dt.uint8
    Alu = mybir.AluOpType
    Act = mybir.ActivationFunctionType

    nc = bacc.Bacc("TRN2", target_bir_lowering=False, debug=False,
                   num_devices=c["cores"])
    cores = list(range(c["cores"]))

    tb, cb = c["tb"], c["cb"]
    hc, heads, ncls = c["hc"], c["heads"], c["ncls"]
    trw, mw = c["trw"], c["mw"]
    sp, nwin = c["slice_pad"], c["nwin"]
    tpw, ntiles = sched["tpw"], sched["ntiles"]
    ntot = int(ntiles[0] + ntiles[1])
    lay = pack_layout(c, ntot)
    K, Q = lay["K"], lay["Q"]
    ngx = -(-tb // 2)
    hcb = cb // 2                      # transpose half-batch

    # ---- I/O
    pack = nc.dram_tensor("pack", [lay["R"], 128], i16, kind="ExternalInput")
    out = nc.dram_tensor("out", [sp, ncls], f16, kind="ExternalOutput")

    # ---- internal DRAM
    bounce1 = nc.dram_tensor("bounce1", [sp, trw], f16)
    bounce2 = nc.dram_tensor("bounce2", [sp, trw], f16)
    tspace = "Shared" if c["cores"] > 4 else "Local"
    table1 = nc.dram_tensor("table1", [c["table_rows"], trw], f16, addr_space=tspace)
    table2 = nc.dram_tensor("table2", [c["table_rows"], trw], f16, addr_space=tspace)
    wshb = nc.dram_tensor("wshb", [49, 128], i16)
    wfull = nc.dram_tensor("wfull", [392, 128], i16)

    with tile.TileContext(nc) as tc:
        with (
            tc.tile_pool(name="const", bufs=1) as constp,
            tc.tile_pool(name="rec", bufs=1) as recp,
            tc.tile_pool(name="accs", bufs=1) as accsp,
            tc.tile_pool(name="big", bufs=2) as bigp,
            tc.tile_pool(name="oh", bufs=2) as ohp,
            tc.tile_pool(name="oht", bufs=2) as ohtp,
            tc.tile_pool(name="small", bufs=2) as smallp,
            tc.tile_pool(name="work", bufs=2) as workp,
            tc.tile_pool(name="psA", bufs=1, space="PSUM") as psA,
            tc.tile_pool(name="psH", bufs=2, space="PSUM") as psH,
            tc.tile_pool(name="psW", bufs=2, space="PSUM") as psW,
            tc.tile_pool(name="psT2", bufs=2, space="PSUM") as psT2,
            tc.tile_pool(name="psAD", bufs=1, space="PSUM") as psAD,
        ):
            # ---------------- constants ----------------
            identF = constp.tile([128, 128], f16, tag="identF")
            make_identity(nc, identF[:])

            # reassemble the shared weight block (each core uploaded 1/8th);
            # collectives cannot read IO tensors, so bounce through DRAM
            nc.sync.dma_start(wshb[:], pack[lay["wsh"]:lay["wsh"] + 49, :])
            nc.gpsimd.collective_compute(
                "AllGather", mybir.AluOpType.bypass, replica_groups=[cores],
                ins=[wshb[:]], outs=[wfull[:]],
            )
            W1s = constp.tile([128, hc], f16, tag="W1s")
            nc.sync.dma_start(W1s[:], wfull[0:128, :].bitcast(f16))
            W2s = constp.tile([128, hc], f16, tag="W2s")
            nc.sync.dma_start(W2s[:], wfull[128:256, :].bitcast(f16))
            miscS = constp.tile([128, 128], f16, tag="miscS")
            nc.sync.dma_start(miscS[:], wfull[256:384, :].bitcast(f16))
            rowsS = []
            for i in range(4):
                rS = constp.tile([1, 128], f16, tag=f"row{i}S")
                nc.sync.dma_start(
                    rS[:], wfull[384 + i:385 + i, :].bitcast(f16))
                rowsS.append(rS)
            onesr = constp.tile([1, 128], f16, tag="onesr")
            nc.vector.memset(onesr[:], 1.0)

            # broadcast single rows across partitions: rank-1 PE matmul
            def bcast_row(row_ap, width, tag, dt):
                ps = psA.tile([128, 128], f32, tag="psA")
                nc.tensor.matmul(out=ps[:, 0:width], lhsT=onesr[:],
                                 rhs=row_ap, start=True, stop=True)
                t = constp.tile([128, width], dt, tag=tag)
                nc.scalar.activation(out=t[:], in_=ps[:, 0:width], func=Act.Copy)
                return t

            iotaS = bcast_row(rowsS[0][:], 128, "iotaS", f16)
            b1s = bcast_row(rowsS[1][:], hc, "b1s", f16)
            b2s = bcast_row(rowsS[2][:], hc, "b2s", f16)
            bouts = bcast_row(rowsS[3][:, 0:ncls], ncls, "bouts", f32)

            # gather index tables: load [16, K*128] and replicate to 128 parts
            gidxS = constp.tile([128, K * 128], i16, tag="gidxS")
            gsrc = pack[lay["gidx"]:lay["gidx"] + 16 * K, :].rearrange(
                "(p k) w -> p (k w)", p=16)
            for k in range(8):
                nc.sync.dma_start(gidxS[16 * k:16 * (k + 1), :], gsrc)

            dstoff8 = constp.tile([128, Q * 256], u8, tag="dstoff8")
            for q in range(Q):
                nc.sync.dma_start(
                    dstoff8[:, q * 256:(q + 1) * 256],
                    pack[lay["dstoff"] + q * 128:lay["dstoff"] + (q + 1) * 128, :]
                    .bitcast(u8))
            dstoffS = constp.tile([128, ntot], f16, tag="dstoffS")
            nc.vector.tensor_copy(out=dstoffS[:], in_=dstoff8[:, 0:ntot])

            # int8 x tiles: one DMA for the whole section
            xq8all = constp.tile([128, ngx, 256], i8, tag="xq8all")
            nc.sync.dma_start(
                xq8all[:],
                pack[lay["xT8"]:lay["xT8"] + ngx * 128, :].bitcast(i8).rearrange(
                    "(g p) w -> p g w", p=128))

            # per-node dequant scales from the spare half of the last xT8 group
            sc16 = constp.tile([128, tb], f16, tag="sc16")
            r0 = lay["xT8"] + (tb // 2) * 128
            nc.sync.dma_start(sc16[:], pack[r0:r0 + 128, 64:64 + tb].bitcast(f16))
            sF = constp.tile([128, tb], f32, tag="sF")
            nc.vector.tensor_copy(out=sF[:], in_=sc16[:])

            accS = accsp.tile([128, tb, mw], f32, tag="accS")

            # ---------------- record-slice build ----------------
            def build_records(get_lhsT, W, WA, rec, scale=None):
                nc.vector.memset(rec[:], 0.0)
                for t in range(tb):
                    lt = get_lhsT(t)
                    h_p = psH.tile([128, hc + 16], f32, tag="psH")
                    nc.tensor.matmul(out=h_p[:, 0:hc], lhsT=lt, rhs=W,
                                     start=True, stop=True)
                    nc.tensor.matmul(out=h_p[:, hc:hc + 16], lhsT=lt, rhs=WA,
                                     start=True, stop=True)
                    if scale is None:
                        nc.scalar.activation(out=rec[:, t, 0:hc + 16],
                                             in_=h_p[:], func=Act.Copy)
                    else:
                        nc.vector.tensor_scalar(
                            out=rec[:, t, 0:hc + 16], in0=h_p[:],
                            scalar1=scale[:, t:t + 1], scalar2=None, op0=Alu.mult)

            def publish(rec, bounce, table):
                nc.sync.dma_start(
                    bounce[:].rearrange("(p t) w -> p t w", p=128), rec[:]
                )
                nc.gpsimd.collective_compute(
                    "AllGather", mybir.AluOpType.bypass,
                    replica_groups=[cores], ins=[bounce[:]], outs=[table[:]],
                )

            # ---------------- edge phase ----------------
            def init_acc_selfloops(rec):
                """acc = self-loop contribution, from the LOCAL record slice
                (overlaps with the table AllGather)."""
                wl = smallp.tile([128, tb, heads], f16, tag="wl")
                nc.vector.tensor_tensor(
                    out=wl[:], in0=rec[:, :, hc:hc + heads],
                    in1=rec[:, :, hc + heads:hc + 2 * heads], op=Alu.add)
                tmp2 = smallp.tile([128, tb, heads], f16, tag="tmp2")
                nc.gpsimd.tensor_scalar(
                    out=tmp2[:], in0=wl[:], scalar1=0.0,
                    scalar2=-(1.0 - NEG_SLOPE), op0=Alu.min, op1=Alu.mult)
                nc.gpsimd.tensor_tensor(
                    out=wl[:], in0=wl[:], in1=tmp2[:], op=Alu.add)
                nc.scalar.activation(out=wl[:], in_=wl[:], func=Act.Exp)
                nc.vector.tensor_tensor(
                    out=accS[:, :, 0:hc].rearrange("p t (h d) -> p t h d", h=heads),
                    in0=rec[:, :, 0:hc].rearrange("p t (h d) -> p t h d", h=heads),
                    in1=wl[:].unsqueeze(-1).to_broadcast([128, tb, heads, c["hid"]]),
                    op=Alu.mult,
                )
                nc.vector.tensor_copy(out=accS[:, :, hc:hc + heads], in_=wl[:])

            def edge_phase(table, rec):
                init_acc_selfloops(rec)
                if not run_edges:
                    return
                tile_base = 0
                for h in (0, 1):
                    tab_h = table[h * c["half_rows"]:(h + 1) * c["half_rows"], :]
                    nt_h = int(ntiles[h])
                    nq = nt_h // cb
                    wins = []
                    twin = []                     # tile (within half) -> window
                    t0 = 0
                    for w in range(nwin):
                        tcnt = int(tpw[h, w])
                        if tcnt:
                            wins.append((w, t0, tcnt))
                            twin.extend([w] * tcnt)
                            t0 += tcnt
                    assert t0 == nt_h
                    widx = 0
                    psw = None
                    for q in range(nq):
                        gg = tile_base + q * cb
                        grec = bigp.tile([128, cb, trw], f16, tag="grec")
                        ccol = gg * 8
                        nc.gpsimd.dma_gather(
                            out_ap=grec[:], in_ap=tab_h,
                            idxs_ap=gidxS[:, ccol:ccol + cb * 8],
                            num_idxs=cb * 128, num_idxs_reg=cb * 128,
                            elem_size=trw,
                        )
                        # batched one-hots: oh[slot, b, row] = (iota==dstoff)
                        oh = ohp.tile([128, cb, 128], f16, tag="oh")
                        nc.vector.tensor_tensor(
                            out=oh[:],
                            in0=iotaS[:].unsqueeze(1).to_broadcast([128, cb, 128]),
                            in1=dstoffS[:, gg:gg + cb].unsqueeze(-1)
                                .to_broadcast([128, cb, 128]),
                            op=Alu.is_equal,
                        )
                        # transposed one-hots: PE transposes in half-batches,
                        # Act engine copies PSUM->SBUF
                        ohTs = ohtp.tile([128, cb, 128], f16, tag="ohT")
                        for hb in range(2):
                            psT = psT2.tile([128, hcb, 128], f16, tag="psT")
                            for b in range(hcb):
                                nc.tensor.transpose(
                                    out=psT[:, b, :],
                                    in_=oh[:, hb * hcb + b, :],
                                    identity=identF[:])
                            nc.scalar.activation(
                                out=ohTs[:, hb * hcb:(hb + 1) * hcb, :],
                                in_=psT[:], func=Act.Copy)
                        # per-edge a_dst via matmul against local records
                        ps_ad = psAD.tile([128, cb, heads], f32, tag="psad")
                        for b in range(cb):
                            wb = twin[q * cb + b]
                            nc.tensor.matmul(
                                out=ps_ad[:, b, :], lhsT=ohTs[:, b, :],
                                rhs=rec[:, wb, hc + heads:hc + 2 * heads],
                                start=True, stop=True,
                            )
                        adh = smallp.tile([128, cb, heads], f16, tag="adh")
                        nc.scalar.activation(out=adh[:], in_=ps_ad[:],
                                             func=Act.Copy)
                        wv = smallp.tile([128, cb, heads], f16, tag="wv")
                        nc.vector.tensor_tensor(
                            out=wv[:], in0=grec[:, :, hc:hc + heads],
                            in1=adh[:], op=Alu.add,
                        )
                        tmp = smallp.tile([128, cb, heads], f16, tag="tmp")
                        nc.gpsimd.tensor_scalar(
                            out=tmp[:], in0=wv[:], scalar1=0.0,
                            scalar2=-(1.0 - NEG_SLOPE), op0=Alu.min, op1=Alu.mult)
                        nc.gpsimd.tensor_tensor(
                            out=wv[:], in0=wv[:], in1=tmp[:], op=Alu.add,
                        )
                        # w = exp(e) written into the record's as-columns
                        nc.scalar.activation(
                            out=grec[:, :, hc:hc + heads], in_=wv[:], func=Act.Exp)
                        # scale h by w per head
                        nc.vector.tensor_tensor(
                            out=grec[:, :, 0:hc].rearrange(
                                "p b (h d) -> p b h d", h=heads),
                            in0=grec[:, :, 0:hc].rearrange(
                                "p b (h d) -> p b h d", h=heads),
                            in1=grec[:, :, hc:hc + heads].unsqueeze(-1)
                                .to_broadcast([128, cb, heads, c["hid"]]),
                            op=Alu.mult,
                        )
                        for b in range(cb):
                            g_h = q * cb + b
                            w, t0w, tcnt = wins[widx]
                            if g_h == t0w:
                                psw = psW.tile([128, mw], f32, tag="psw")
                            nc.tensor.matmul(
                                out=psw[:], lhsT=oh[:, b, :],
                                rhs=grec[:, b, 0:mw],
                                start=g_h == t0w, stop=g_h == t0w + tcnt - 1,
                            )
                            if g_h == t0w + tcnt - 1:
                                nc.vector.tensor_tensor(
                                    out=accS[:, w, :], in0=accS[:, w, :],
                                    in1=psw[:], op=Alu.add,
                                )
                                widx += 1
                    tile_base += nt_h

            # ---------------- divide + bias + relu ----------------
            def finish_layer(bias, ytile):
                rcp = smallp.tile([128, tb, heads], f32, tag="rcp")
                nc.vector.tensor_scalar(
                    out=rcp[:], in0=accS[:, :, hc:hc + heads],
                    scalar1=1e-9, scalar2=None, op0=Alu.add,
                )
                nc.vector.reciprocal(out=rcp[:], in_=rcp[:])
                nc.vector.tensor_tensor(
                    out=ytile[:].rearrange("p t (h d) -> p t h d", h=heads),
                    in0=accS[:, :, 0:hc].rearrange("p t (h d) -> p t h d", h=heads),
                    in1=rcp[:].unsqueeze(-1).to_broadcast([128, tb, heads, c["hid"]]),
                    op=Alu.mult,
                )
                nc.vector.tensor_tensor(
                    out=ytile[:], in0=ytile[:],
                    in1=bias.unsqueeze(1).to_broadcast([128, tb, hc]),
                    op=Alu.add,
                )
                nc.vector.tensor_scalar(
                    out=ytile[:], in0=ytile[:], scalar1=0.0, scalar2=None,
                    op0=Alu.max,
                )

            # lhsT providers: layer 1 reads pre-transposed int8 tiles from
            # SBUF; later layers transpose on-device and downcast to f16.
            def x_lhsT(t):
                g, hb = t // 2, t % 2
                xt = workp.tile([128, 128], f16, tag="xt")
                nc.vector.tensor_copy(
                    out=xt[:], in_=xq8all[:, g, 128 * hb:128 * (hb + 1)])
                return xt[:]

            def make_y_lhsT(ytile):
                def y_lhsT(t):
                    yT_p = psA.tile([128, 128], f16, tag="psA")
                    nc.tensor.transpose(out=yT_p[:], in_=ytile[:, t, :],
                                        identity=identF[:])
                    yTs = workp.tile([128, 128], f16, tag="xt")
                    nc.scalar.activation(out=yTs[:], in_=yT_p[:], func=Act.Copy)
                    return yTs[:]
                return y_lhsT

            # ================ layer 1 ================
            rec1 = recp.tile([128, tb, trw], f16, tag="rec")
            build_records(x_lhsT, W1s[:], miscS[:, 0:16], rec1, scale=sF)
            publish(rec1, bounce1, table1)
            edge_phase(table1, rec1)
            y1 = recp.tile([128, tb, hc], f16, tag="y")
            finish_layer(b1s[:], y1)

            # ================ layer 2 ================
            rec2 = recp.tile([128, tb, trw], f16, tag="rec")
            build_records(make_y_lhsT(y1), W2s[:], miscS[:, 16:32], rec2)
            publish(rec2, bounce2, table2)
            edge_phase(table2, rec2)
            y2 = recp.tile([128, tb, hc], f16, tag="y")
            finish_layer(b2s[:], y2)

            # ================ output projection ================
            outt = recp.tile([128, tb, ncls], f16, tag="outt")
            y_lhsT2 = make_y_lhsT(y2)
            for t in range(tb):
                yTs = y_lhsT2(t)
                o_p = psH.tile([128, hc + 16], f32, tag="psH")
                nc.tensor.matmul(out=o_p[:, 0:ncls], lhsT=yTs,
                                 rhs=miscS[:, 32:32 + ncls],
                                 start=True, stop=True)
                nc.vector.tensor_tensor(
                    out=outt[:, t, :], in0=o_p[:, 0:ncls], in1=bouts[:],
                    op=Alu.add,
                )
            nc.sync.dma_start(
                out[:].rearrange("(p t) w -> p t w", p=128), outt[:]
            )

    nc.compile()
    return nc


# ---------------------------------------------------------------- entry point

_CACHE = {}


def prepare(inputs, c):
    """inputs dict -> (in_maps, sched, perm)."""
    x = np.asarray(inputs["x"], np.float32)
    edge_index = np.asarray(inputs["edge_index"])
    edge_maps, sched, perm = host_prep(x, edge_index, c)
    packs = host_pack(
        x, edge_maps, sched, perm, inputs["W1"], inputs["a_src1"],
        inputs["a_dst1"], inputs["b1"], inputs["W2"], inputs["a_src2"],
        inputs["a_dst2"], inputs["b2"], inputs["Wout"], inputs["bout"], c)
    in_maps = [dict(pack=p) for p in packs]
    return in_maps, sched, perm


def kernel(x, edge_index, W1, a_src1, a_dst1, b1, W2, a_src2, a_dst2, b2,
           Wout, bout):
    from concourse.bass_utils import run_bass_kernel_spmd

    c = derive(full_cfg())
    in_maps, sched, perm = prepare(dict(
        x=x, edge_index=edge_index, W1=W1, a_src1=a_src1, a_dst1=a_dst1,
        b1=b1, W2=W2, a_src2=a_src2, a_dst2=a_dst2, b2=b2, Wout=Wout,
        bout=bout), c)
    key = ("full", sched["tpw"].tobytes())
    if key not in _CACHE:
        _CACHE[key] = build_nc(c, sched)
    nc = _CACHE[key]
    res = run_bass_kernel_spmd(nc, in_maps, list(range(c["cores"])))
    return host_post(res.results, perm, c)


# revision 39
# speedup vs baseline: 1.0297x; 1.0297x over previous
"""GAT (2-layer, 8-head) Bass kernel for 8 Trainium2 NeuronCores.

Strategy (edge-parallel, dst-sharded):
  - Nodes are assigned to 8 cores x 49 windows of 128 slots by a
    degree-balanced permutation (computed on host), so every window carries
    ~equal edge count and the shared SPMD tile schedule has minimal padding.
  - Core c owns its 49 windows (6272 node slots) and processes all edges
    whose dst lands there.  Per 128-edge tile a one-hot (edge x window-row)
    matrix is built and a PE matmul accumulates weighted messages into a
    PSUM window; per-edge softmax weight w = exp(leaky_relu(as[src]+ad[dst]))
    with as gathered with the src record and ad fetched via a matmul against
    the transposed one-hot.  Divide + bias + relu at node level; repeat for
    layer 2; output projection.
  - Records are float16, 256 wide (h 128 | as 8 | ad 8 | pad): the
    dma_gather moves 512B/edge and the AllGather of the replicated node
    table moves 3.2MB/core/layer.
  - Work is spread across engines: PE (matmuls, one-hot transposes),
    DVE (one-hot build, message scaling), Act (PSUM->SBUF batch copies,
    exp), Pool/GPSIMD (gathers, leaky-relu), so the edge phase overlaps.

  Transfer format (the axon tunnel dominates end-to-end wall time, so all
  per-core inputs are packed into ONE int16 tensor, ~1.15 MB/core):
  - x ships as int8, quantized per node (scale = max|x_row|/127, f16 scales
    alongside), pre-transposed into 128x128 lhsT tiles.
  - Weights ship as float16 (W1, W2, W@A_blockdiag fused on host, Wout);
    each core carries 1/8th, reassembled on device by a small AllGather.
  - dma_gather index table ships 16-partition-wrapped; dstoff (edge ->
    destination row-in-window) ships as uint8 (255 = pad).
  - The output ships back as float16.
"""

import sys
import os

for _p in ("/opt/trn_rl_repo", "/root/.axon_site/_ro/trn_rl_repo"):
    if os.path.isdir(_p) and _p not in sys.path:
        sys.path.insert(0, _p)

import numpy as np

NEG_SLOPE = 0.2
WW = 128      # window rows = one 128-node block (partition-aligned)


def full_cfg():
    return dict(cores=8, n=50000, tb=49, cb=8, in_ch=128, hc=128,
                heads=8, hid=16, ncls=10)


def derive(cfg):
    d = dict(cfg)
    d["slice_pad"] = d["tb"] * 128              # node slots per core
    d["table_rows"] = d["cores"] * d["slice_pad"]
    d["half_rows"] = d["table_rows"] // 2
    d["trw"] = 256                     # table row width (f16)
    d["mw"] = d["hc"] + d["heads"]     # message width: h|w
    d["chunk"] = 128 * d["cb"]
    d["nwin"] = d["tb"]
    return d


def pack_layout(c, ntot):
    """Row offsets into the per-core [R, 128] int16 pack tensor."""
    K = -(-(ntot * 8) // 128)      # columns (of 128 i16) for idx tables
    Q = -(-ntot // 256)            # 128-row groups for the u8 dstoff table
    ngx = -(-c["tb"] // 2)         # int8 xT tile pairs
    off = {}
    r = 0
    off["xT8"] = r; r += ngx * 128   # i8: tile 2g in bytes 0:128, 2g+1 in 128:256
    off["gidx"] = r; r += 16 * K
    off["dstoff"] = r; r += Q * 128  # u8 [128, ntot] (255 = pad slot)
    # weights are identical on every core: each shard carries 1/8th of the
    # 392-row block (W1 | W2 | misc | rows4 | pad); an on-device AllGather
    # reassembles it. Full-block layout: 0:128 W1, 128:256 W2, 256:384 misc
    # (f16 cols 0:16 WA1 | 16:32 WA2 | 32:42 Wout | 42:42+tb x-scales),
    # 384:388 rows4 (iota | b1 | b2 | bout).
    off["wsh"] = r; r += 392 // 8
    off["R"] = r
    off["K"] = K
    off["Q"] = Q
    return off


# ---------------------------------------------------------------- host prep

def balance_nodes(edge_index, c):
    """Degree-balanced node -> (core, window, pos) assignment.

    Returns (node_core, node_win, node_pos): per-node core id, window id
    within the core (0..tb-1), and row within the window (0..127).
    """
    import heapq
    n, cores, nwin = c["n"], c["cores"], c["nwin"]
    # self-loops are handled node-locally on device, so balance on indegree
    deg = np.bincount(np.asarray(edge_index[1], np.int64) % n,
                      minlength=n).astype(np.int64)
    nw = cores * nwin
    order = np.argsort(-deg, kind="stable")
    heap = [(0, w) for w in range(nw)]
    heapq.heapify(heap)
    count = np.zeros(nw, np.int32)
    node_w = np.zeros(n, np.int32)
    node_pos = np.zeros(n, np.int32)
    for nid in order:
        while True:
            load, w = heapq.heappop(heap)
            if count[w] < WW:
                break
        node_w[nid] = w
        node_pos[nid] = count[w]
        count[w] += 1
        if count[w] < WW:
            heapq.heappush(heap, (load + int(deg[nid]), w))
    return node_w // nwin, node_w % nwin, node_pos


def host_prep(x, edge_index, c):
    """Node permutation + per-core edge maps + shared window schedule."""
    n, cores = c["n"], c["cores"]
    tb, cb, nwin = c["tb"], c["cb"], c["nwin"]
    sp = c["slice_pad"]

    node_core, node_win, node_pos = balance_nodes(edge_index, c)
    # table row of each node (within the full 8-core table)
    trow_n = node_core * sp + node_pos * tb + node_win

    # self-loops (PyG GATConv default) are handled node-locally on device;
    # only the real edges go through the gather/one-hot machinery.
    src = np.asarray(edge_index[0], np.int64)
    dst = np.asarray(edge_index[1], np.int64)
    trow = trow_n[src]
    half = (trow >= c["half_rows"]).astype(np.int64)
    owner = node_core[dst]
    win = node_win[dst]
    pos = node_pos[dst]

    # counts per (core, half, window) in one bincount
    key = (owner * 2 + half) * nwin + win
    counts = np.bincount(key, minlength=cores * 2 * nwin).reshape(cores, 2, nwin)
    tpw = -(-counts.max(axis=0) // 128)          # [2, nwin] tiles per window
    ntiles = tpw.sum(axis=1)
    for h in (0, 1):
        padt = (-int(ntiles[h])) % cb
        if padt:
            nz = np.nonzero(tpw[h])[0]
            wlast = int(nz[-1]) if len(nz) else 0
            tpw[h, wlast] += padt
            ntiles[h] += padt
    sched = dict(tpw=tpw, ntiles=[int(ntiles[0]), int(ntiles[1])])

    ntot = int(ntiles.sum())
    cap = ntot * 128
    # tile start (global, across both halves) for each (half, window)
    tstart = np.zeros((2, nwin), np.int64)
    tstart[0] = np.cumsum(tpw[0]) - tpw[0]
    tstart[1] = int(ntiles[0]) + np.cumsum(tpw[1]) - tpw[1]

    maps = []
    for core in range(cores):
        m = owner == core
        tr_c = trow[m]
        wn_c = win[m]
        ps_c = pos[m]
        hf_c = half[m]
        order = np.lexsort((wn_c, hf_c))
        tr_c, wn_c, ps_c, hf_c = tr_c[order], wn_c[order], ps_c[order], hf_c[order]

        bkey = hf_c * nwin + wn_c
        bcnt = np.bincount(bkey, minlength=2 * nwin)
        bstart = np.cumsum(bcnt) - bcnt             # start idx in sorted order
        rank = np.arange(len(bkey)) - bstart[bkey]
        base = (tstart.reshape(-1)[bkey]) * 128
        slots = base + rank
        assert np.all(rank < tpw.reshape(-1)[bkey] * 128)

        srcrow = np.zeros(cap, np.int64)            # pads: row 0
        dstoff = np.full(cap, 255, np.uint8)        # pads: no one-hot match
        srcrow[slots] = tr_c - hf_c * c["half_rows"]
        dstoff[slots] = ps_c.astype(np.uint8)

        def wrap16(vals):
            nq = ntot // cb
            v = vals.reshape(nq, cb * 128)
            w16 = np.zeros((nq, 16, cb * 8), np.int16)
            k = np.arange(cb * 128)
            w16[:, k % 16, k // 16] = v
            return w16.transpose(1, 0, 2).reshape(16, nq * cb * 8)

        maps.append(dict(
            gidx=wrap16(srcrow.astype(np.int16)),
            dstoff=np.ascontiguousarray(dstoff.reshape(ntot, 128).T),
        ))
    perm = dict(core=node_core, win=node_win, pos=node_pos)
    return maps, sched, perm


def host_pack(x, edge_maps, sched, perm, W1, a_src1, a_dst1, b1, W2, a_src2,
              a_dst2, b2, Wout, bout, c):
    """Assemble the per-core [R, 128] int16 pack tensors."""
    heads, hid, hc, tb = c["heads"], c["hid"], c["hc"], c["tb"]
    sp = c["slice_pad"]
    ntot = int(sched["ntiles"][0] + sched["ntiles"][1])
    lay = pack_layout(c, ntot)

    def blockdiag(a):
        A = np.zeros((hc, heads), np.float32)
        for h in range(heads):
            A[h * hid:(h + 1) * hid, h] = a[h]
        return A

    W1 = np.asarray(W1, np.float32)
    W2 = np.asarray(W2, np.float32)
    WA1 = np.concatenate([W1 @ blockdiag(np.asarray(a_src1, np.float32)),
                          W1 @ blockdiag(np.asarray(a_dst1, np.float32))], axis=1)
    WA2 = np.concatenate([W2 @ blockdiag(np.asarray(a_src2, np.float32)),
                          W2 @ blockdiag(np.asarray(a_dst2, np.float32))], axis=1)
    misc = np.zeros((128, 128), np.float16)
    misc[:, 0:16] = WA1.astype(np.float16)
    misc[:, 16:32] = WA2.astype(np.float16)
    misc[:, 32:42] = np.asarray(Wout, np.float32).astype(np.float16)
    rows4 = np.zeros((4, 128), np.float16)
    rows4[0] = np.arange(128, dtype=np.float16)
    rows4[1] = np.asarray(b1, np.float32).astype(np.float16)
    rows4[2] = np.asarray(b2, np.float32).astype(np.float16)
    rows4[3, 0:c["ncls"]] = np.asarray(bout, np.float32).astype(np.float16)
    wblock = np.zeros((392, 128), np.int16)
    wblock[0:128] = W1.astype(np.float16).view(np.int16)
    wblock[128:256] = W2.astype(np.float16).view(np.int16)
    wblock[256:384] = misc.view(np.int16)
    wblock[384:388] = rows4.view(np.int16)

    x = np.asarray(x, np.float32)
    n = c["n"]
    slot_n = perm["win"].astype(np.int64) * WW + perm["pos"]   # slot in core
    ngx = -(-tb // 2)
    packs = []
    for core in range(c["cores"]):
        em = edge_maps[core]
        p = np.zeros((lay["R"], 128), np.int16)
        xs = np.zeros((sp, c["in_ch"]), np.float32)
        mask = perm["core"] == core
        xs[slot_n[mask]] = x[mask]
        s = np.maximum(np.abs(xs).max(axis=1) / 127.0, 1e-4).astype(np.float16)
        xq = np.round(xs / s[:, None].astype(np.float32)).clip(-127, 127)
        # per-tile transposed lhsT blocks [128 in_ch, 128 nodes], int8
        xT = np.ascontiguousarray(
            xq.astype(np.int8).reshape(tb, 128, c["in_ch"]).transpose(0, 2, 1))
        blocks = np.zeros((ngx, 128, 256), np.int8)
        blocks[:, :, 0:128] = xT[0::2]
        blocks[: tb // 2, :, 128:256] = xT[1::2]
        # per-node dequant scales ride in the unused half of the last
        # (odd-tb) group: [128, tb] f16 as raw bytes
        assert tb % 2 == 1
        blocks[tb // 2, :, 128:128 + 2 * tb] = np.ascontiguousarray(
            s.reshape(tb, 128).T).view(np.int8)
        p[lay["xT8"]:lay["xT8"] + ngx * 128] = \
            blocks.reshape(ngx * 128, 256).view(np.int16)

        K = lay["K"]
        gi = np.zeros((16, K * 128), np.int16)
        gi[:, :ntot * 8] = em["gidx"]
        p[lay["gidx"]:lay["gidx"] + 16 * K] = gi.reshape(16 * K, 128)

        Q = lay["Q"]
        doff = np.full((128, Q * 256), 255, np.uint8)
        doff[:, :ntot] = em["dstoff"]
        p[lay["dstoff"]:lay["dstoff"] + Q * 128] = np.ascontiguousarray(
            doff.reshape(128, Q, 256).transpose(1, 0, 2)
        ).reshape(Q * 128, 256).view(np.int16)

        p[lay["wsh"]:lay["wsh"] + 49] = wblock[core * 49:(core + 1) * 49]
        packs.append(p)
    return packs


def host_post(results, perm, c):
    n, tb = c["n"], c["tb"]
    out = np.zeros((n, c["ncls"]), np.float32)
    rows = perm["pos"].astype(np.int64) * tb + perm["win"]
    for core in range(c["cores"]):
        res = np.asarray(results[core]["out"], np.float32)
        mask = perm["core"] == core
        out[mask] = res[rows[mask]]
    return out


# ---------------------------------------------------------------- device build

def build_nc(c, sched, run_edges=True, tl_mode=False, pub_g=1):
    """tl_mode: single-core build with collectives replaced by local DMAs,
    for TimelineSim-based engine-occupancy analysis (values are garbage).
    pub_g: number of window-groups for chunked publish+AllGather."""
    from concourse import bass, mybir, bacc, tile
    from concourse.masks import make_identity

    f32 = mybir.dt.float32
    f16 = mybir.dt.float16
    i16 = mybir.dt.int16
    i8 = mybir.dt.int8
    u8 = mybir.dt.uint8
    Alu = mybir.AluOpType
    Act = mybir.ActivationFunctionType

    nc = bacc.Bacc("TRN2", target_bir_lowering=False, debug=False,
                   num_devices=1 if tl_mode else c["cores"])
    cores = list(range(c["cores"]))

    def allgather(ins, outs):
        if tl_mode:
            n_in = ins[0].shape[0]
            nc.sync.dma_start(outs[0][0:n_in], ins[0])
        else:
            nc.gpsimd.collective_compute(
                "AllGather", mybir.AluOpType.bypass, replica_groups=[cores],
                ins=ins, outs=outs,
            )

    tb, cb = c["tb"], c["cb"]
    hc, heads, ncls = c["hc"], c["heads"], c["ncls"]
    trw, mw = c["trw"], c["mw"]
    sp, nwin = c["slice_pad"], c["nwin"]
    tpw, ntiles = sched["tpw"], sched["ntiles"]
    ntot = int(ntiles[0] + ntiles[1])
    lay = pack_layout(c, ntot)
    K, Q = lay["K"], lay["Q"]
    ngx = -(-tb // 2)
    hcb = cb // 2                      # transpose half-batch

    # ---- I/O
    pack = nc.dram_tensor("pack", [lay["R"], 128], i16, kind="ExternalInput")
    out = nc.dram_tensor("out", [sp, ncls], f16, kind="ExternalOutput")

    # ---- internal DRAM
    bounce1 = nc.dram_tensor("bounce1", [sp, trw], f16)
    bounce2 = nc.dram_tensor("bounce2", [sp, trw], f16)
    tspace = "Shared" if c["cores"] > 4 else "Local"
    table1 = nc.dram_tensor("table1", [c["table_rows"], trw], f16, addr_space=tspace)
    table2 = nc.dram_tensor("table2", [c["table_rows"], trw], f16, addr_space=tspace)
    wshb = nc.dram_tensor("wshb", [49, 128], i16)
    wfull = nc.dram_tensor("wfull", [392, 128], i16)

    with tile.TileContext(nc) as tc:
        with (
            tc.tile_pool(name="const", bufs=1) as constp,
            tc.tile_pool(name="rec", bufs=1) as recp,
            tc.tile_pool(name="accs", bufs=1) as accsp,
            tc.tile_pool(name="big", bufs=2) as bigp,
            tc.tile_pool(name="oh", bufs=2) as ohp,
            tc.tile_pool(name="oht", bufs=2) as ohtp,
            tc.tile_pool(name="small", bufs=2) as smallp,
            tc.tile_pool(name="work", bufs=2) as workp,
            tc.tile_pool(name="psA", bufs=1, space="PSUM") as psA,
            tc.tile_pool(name="psH", bufs=2, space="PSUM") as psH,
            tc.tile_pool(name="psW", bufs=2, space="PSUM") as psW,
            tc.tile_pool(name="psT2", bufs=2, space="PSUM") as psT2,
            tc.tile_pool(name="psAD", bufs=1, space="PSUM") as psAD,
        ):
            # ---------------- constants ----------------
            identF = constp.tile([128, 128], f16, tag="identF")
            make_identity(nc, identF[:])

            # reassemble the shared weight block (each core uploaded 1/8th);
            # collectives cannot read IO tensors, so bounce through DRAM
            nc.sync.dma_start(wshb[:], pack[lay["wsh"]:lay["wsh"] + 49, :])
            allgather([wshb[:]], [wfull[:]])
            W1s = constp.tile([128, hc], f16, tag="W1s")
            nc.sync.dma_start(W1s[:], wfull[0:128, :].bitcast(f16))
            W2s = constp.tile([128, hc], f16, tag="W2s")
            nc.sync.dma_start(W2s[:], wfull[128:256, :].bitcast(f16))
            miscS = constp.tile([128, 128], f16, tag="miscS")
            nc.sync.dma_start(miscS[:], wfull[256:384, :].bitcast(f16))
            rowsS = []
            for i in range(4):
                rS = constp.tile([1, 128], f16, tag=f"row{i}S")
                nc.sync.dma_start(
                    rS[:], wfull[384 + i:385 + i, :].bitcast(f16))
                rowsS.append(rS)
            onesr = constp.tile([1, 128], f16, tag="onesr")
            nc.vector.memset(onesr[:], 1.0)

            # broadcast single rows across partitions: rank-1 PE matmul
            def bcast_row(row_ap, width, tag, dt):
                ps = psA.tile([128, 128], f32, tag="psA")
                nc.tensor.matmul(out=ps[:, 0:width], lhsT=onesr[:],
                                 rhs=row_ap, start=True, stop=True)
                t = constp.tile([128, width], dt, tag=tag)
                nc.scalar.activation(out=t[:], in_=ps[:, 0:width], func=Act.Copy)
                return t

            iotaS = bcast_row(rowsS[0][:], 128, "iotaS", f16)
            b1s = bcast_row(rowsS[1][:], hc, "b1s", f16)
            b2s = bcast_row(rowsS[2][:], hc, "b2s", f16)
            bouts = bcast_row(rowsS[3][:, 0:ncls], ncls, "bouts", f32)

            # gather index tables: load [16, K*128] and replicate to 128 parts
            gidxS = constp.tile([128, K * 128], i16, tag="gidxS")
            gsrc = pack[lay["gidx"]:lay["gidx"] + 16 * K, :].rearrange(
                "(p k) w -> p (k w)", p=16)
            for k in range(8):
                nc.sync.dma_start(gidxS[16 * k:16 * (k + 1), :], gsrc)

            dstoff8 = constp.tile([128, Q * 256], u8, tag="dstoff8")
            for q in range(Q):
                nc.sync.dma_start(
                    dstoff8[:, q * 256:(q + 1) * 256],
                    pack[lay["dstoff"] + q * 128:lay["dstoff"] + (q + 1) * 128, :]
                    .bitcast(u8))
            dstoffS = constp.tile([128, ntot], f16, tag="dstoffS")
            nc.vector.tensor_copy(out=dstoffS[:], in_=dstoff8[:, 0:ntot])

            # int8 x tiles: one DMA for the whole section
            xq8all = constp.tile([128, ngx, 256], i8, tag="xq8all")
            nc.sync.dma_start(
                xq8all[:],
                pack[lay["xT8"]:lay["xT8"] + ngx * 128, :].bitcast(i8).rearrange(
                    "(g p) w -> p g w", p=128))

            # per-node dequant scales from the spare half of the last xT8 group
            sc16 = constp.tile([128, tb], f16, tag="sc16")
            r0 = lay["xT8"] + (tb // 2) * 128
            nc.sync.dma_start(sc16[:], pack[r0:r0 + 128, 64:64 + tb].bitcast(f16))
            sF = constp.tile([128, tb], f32, tag="sF")
            nc.vector.tensor_copy(out=sF[:], in_=sc16[:])

            accS = accsp.tile([128, tb, mw], f32, tag="accS")

            # ---------------- record-slice build ----------------
            # publish in PUB_G window-groups so the AllGather of group g
            # overlaps the record build of group g+1 (1 = single flat publish)
            PUB_G = pub_g

            def pub_groups():
                base = tb // PUB_G
                rem = tb % PUB_G
                t0 = 0
                for g in range(PUB_G):
                    n = base + (1 if g < rem else 0)
                    yield t0, t0 + n
                    t0 += n

            def build_publish(get_lhsT, W, WA, rec, bounce, table, scale=None):
                nc.vector.memset(rec[:], 0.0)
                b3 = bounce[:].rearrange("(p t) w -> p t w", p=128)
                t4 = table[:].rearrange("(r p t) w -> r p t w", r=c["cores"],
                                        p=128)
                t03 = table[0:sp].rearrange("(p t) w -> p t w", p=128)
                for w0, w1 in pub_groups():
                    for t in range(w0, w1):
                        lt = get_lhsT(t)
                        h_p = psH.tile([128, hc + 16], f32, tag="psH")
                        nc.tensor.matmul(out=h_p[:, 0:hc], lhsT=lt, rhs=W,
                                         start=True, stop=True)
                        nc.tensor.matmul(out=h_p[:, hc:hc + 16], lhsT=lt,
                                         rhs=WA, start=True, stop=True)
                        if scale is None:
                            nc.scalar.activation(out=rec[:, t, 0:hc + 16],
                                                 in_=h_p[:], func=Act.Copy)
                        else:
                            nc.vector.tensor_scalar(
                                out=rec[:, t, 0:hc + 16], in0=h_p[:],
                                scalar1=scale[:, t:t + 1], scalar2=None,
                                op0=Alu.mult)
                    if PUB_G == 1:
                        nc.sync.dma_start(b3[:], rec[:])
                        allgather([bounce[:]], [table[:]])
                        continue
                    nc.sync.dma_start(b3[:, w0:w1, :], rec[:, w0:w1, :])
                    if tl_mode:
                        nc.sync.dma_start(t03[:, w0:w1, :], b3[:, w0:w1, :])
                    else:
                        nc.gpsimd.collective_compute(
                            "AllGather", mybir.AluOpType.bypass,
                            replica_groups=[cores],
                            ins=[b3[:, w0:w1, :]], outs=[t4[:, :, w0:w1, :]],
                        )

            # ---------------- edge phase ----------------
            def init_acc_selfloops(rec):
                """acc = self-loop contribution, from the LOCAL record slice
                (overlaps with the table AllGather)."""
                wl = smallp.tile([128, tb, heads], f16, tag="wl")
                nc.vector.tensor_tensor(
                    out=wl[:], in0=rec[:, :, hc:hc + heads],
                    in1=rec[:, :, hc + heads:hc + 2 * heads], op=Alu.add)
                tmp2 = smallp.tile([128, tb, heads], f16, tag="tmp2")
                nc.gpsimd.tensor_scalar(
                    out=tmp2[:], in0=wl[:], scalar1=0.0,
                    scalar2=-(1.0 - NEG_SLOPE), op0=Alu.min, op1=Alu.mult)
                nc.gpsimd.tensor_tensor(
                    out=wl[:], in0=wl[:], in1=tmp2[:], op=Alu.add)
                nc.scalar.activation(out=wl[:], in_=wl[:], func=Act.Exp)
                nc.vector.tensor_tensor(
                    out=accS[:, :, 0:hc].rearrange("p t (h d) -> p t h d", h=heads),
                    in0=rec[:, :, 0:hc].rearrange("p t (h d) -> p t h d", h=heads),
                    in1=wl[:].unsqueeze(-1).to_broadcast([128, tb, heads, c["hid"]]),
                    op=Alu.mult,
                )
                nc.vector.tensor_copy(out=accS[:, :, hc:hc + heads], in_=wl[:])

            def edge_phase(table, rec):
                init_acc_selfloops(rec)
                if not run_edges:
                    return
                tile_base = 0
                for h in (0, 1):
                    tab_h = table[h * c["half_rows"]:(h + 1) * c["half_rows"], :]
                    nt_h = int(ntiles[h])
                    nq = nt_h // cb
                    wins = []
                    twin = []                     # tile (within half) -> window
                    t0 = 0
                    for w in range(nwin):
                        tcnt = int(tpw[h, w])
                        if tcnt:
                            wins.append((w, t0, tcnt))
                            twin.extend([w] * tcnt)
                            t0 += tcnt
                    assert t0 == nt_h
                    widx = 0
                    psw = None
                    for q in range(nq):
                        gg = tile_base + q * cb
                        grec = bigp.tile([128, cb, trw], f16, tag="grec")
                        ccol = gg * 8
                        nc.gpsimd.dma_gather(
                            out_ap=grec[:], in_ap=tab_h,
                            idxs_ap=gidxS[:, ccol:ccol + cb * 8],
                            num_idxs=cb * 128, num_idxs_reg=cb * 128,
                            elem_size=trw,
                        )
                        # batched one-hots: oh[slot, b, row] = (iota==dstoff)
                        # (is_equal is not in the Pool engine's ISA: DVE only)
                        oh = ohp.tile([128, cb, 128], f16, tag="oh")
                        nc.vector.tensor_tensor(
                            out=oh[:],
                            in0=iotaS[:].unsqueeze(1).to_broadcast([128, cb, 128]),
                            in1=dstoffS[:, gg:gg + cb].unsqueeze(-1)
                                .to_broadcast([128, cb, 128]),
                            op=Alu.is_equal,
                        )
                        # transposed one-hots: PE transposes in half-batches,
                        # Act engine copies PSUM->SBUF
                        ohTs = ohtp.tile([128, cb, 128], f16, tag="ohT")
                        for hb in range(2):
                            psT = psT2.tile([128, hcb, 128], f16, tag="psT")
                            for b in range(hcb):
                                nc.tensor.transpose(
                                    out=psT[:, b, :],
                                    in_=oh[:, hb * hcb + b, :],
                                    identity=identF[:])
                            nc.scalar.activation(
                                out=ohTs[:, hb * hcb:(hb + 1) * hcb, :],
                                in_=psT[:], func=Act.Copy)
                        # per-edge a_dst via matmul against local records
                        ps_ad = psAD.tile([128, cb, heads], f32, tag="psad")
                        for b in range(cb):
                            wb = twin[q * cb + b]
                            nc.tensor.matmul(
                                out=ps_ad[:, b, :], lhsT=ohTs[:, b, :],
                                rhs=rec[:, wb, hc + heads:hc + 2 * heads],
                                start=True, stop=True,
                            )
                        adh = smallp.tile([128, cb, heads], f16, tag="adh")
                        nc.scalar.activation(out=adh[:], in_=ps_ad[:],
                                             func=Act.Copy)
                        wv = smallp.tile([128, cb, heads], f16, tag="wv")
                        nc.vector.tensor_tensor(
                            out=wv[:], in0=grec[:, :, hc:hc + heads],
                            in1=adh[:], op=Alu.add,
                        )
                        tmp = smallp.tile([128, cb, heads], f16, tag="tmp")
                        nc.gpsimd.tensor_scalar(
                            out=tmp[:], in0=wv[:], scalar1=0.0,
                            scalar2=-(1.0 - NEG_SLOPE), op0=Alu.min, op1=Alu.mult)
                        nc.gpsimd.tensor_tensor(
                            out=wv[:], in0=wv[:], in1=tmp[:], op=Alu.add,
                        )
                        # w = exp(e) written into the record's as-columns
                        nc.scalar.activation(
                            out=grec[:, :, hc:hc + heads], in_=wv[:], func=Act.Exp)
                        # scale h by w per head
                        nc.vector.tensor_tensor(
                            out=grec[:, :, 0:hc].rearrange(
                                "p b (h d) -> p b h d", h=heads),
                            in0=grec[:, :, 0:hc].rearrange(
                                "p b (h d) -> p b h d", h=heads),
                            in1=grec[:, :, hc:hc + heads].unsqueeze(-1)
                                .to_broadcast([128, cb, heads, c["hid"]]),
                            op=Alu.mult,
                        )
                        for b in range(cb):
                            g_h = q * cb + b
                            w, t0w, tcnt = wins[widx]
                            if g_h == t0w:
                                psw = psW.tile([128, mw], f32, tag="psw")
                            nc.tensor.matmul(
                                out=psw[:], lhsT=oh[:, b, :],
                                rhs=grec[:, b, 0:mw],
                                start=g_h == t0w, stop=g_h == t0w + tcnt - 1,
                            )
                            if g_h == t0w + tcnt - 1:
                                nc.vector.tensor_tensor(
                                    out=accS[:, w, :], in0=accS[:, w, :],
                                    in1=psw[:], op=Alu.add,
                                )
                                widx += 1
                    tile_base += nt_h

            # ---------------- divide + bias + relu ----------------
            def finish_layer(bias, ytile):
                rcp = smallp.tile([128, tb, heads], f32, tag="rcp")
                nc.vector.tensor_scalar(
                    out=rcp[:], in0=accS[:, :, hc:hc + heads],
                    scalar1=1e-9, scalar2=None, op0=Alu.add,
                )
                nc.vector.reciprocal(out=rcp[:], in_=rcp[:])
                nc.vector.tensor_tensor(
                    out=ytile[:].rearrange("p t (h d) -> p t h d", h=heads),
                    in0=accS[:, :, 0:hc].rearrange("p t (h d) -> p t h d", h=heads),
                    in1=rcp[:].unsqueeze(-1).to_broadcast([128, tb, heads, c["hid"]]),
                    op=Alu.mult,
                )
                nc.vector.tensor_tensor(
                    out=ytile[:], in0=ytile[:],
                    in1=bias.unsqueeze(1).to_broadcast([128, tb, hc]),
                    op=Alu.add,
                )
                nc.vector.tensor_scalar(
                    out=ytile[:], in0=ytile[:], scalar1=0.0, scalar2=None,
                    op0=Alu.max,
                )

            # lhsT providers: layer 1 reads pre-transposed int8 tiles from
            # SBUF; later layers transpose on-device and downcast to f16.
            def x_lhsT(t):
                g, hb = t // 2, t % 2
                xt = workp.tile([128, 128], f16, tag="xt")
                nc.vector.tensor_copy(
                    out=xt[:], in_=xq8all[:, g, 128 * hb:128 * (hb + 1)])
                return xt[:]

            def make_y_lhsT(ytile):
                def y_lhsT(t):
                    yT_p = psA.tile([128, 128], f16, tag="psA")
                    nc.tensor.transpose(out=yT_p[:], in_=ytile[:, t, :],
                                        identity=identF[:])
                    yTs = workp.tile([128, 128], f16, tag="xt")
                    nc.scalar.activation(out=yTs[:], in_=yT_p[:], func=Act.Copy)
                    return yTs[:]
                return y_lhsT

            # ================ layer 1 ================
            rec1 = recp.tile([128, tb, trw], f16, tag="rec")
            build_publish(x_lhsT, W1s[:], miscS[:, 0:16], rec1, bounce1,
                          table1, scale=sF)
            edge_phase(table1, rec1)
            y1 = recp.tile([128, tb, hc], f16, tag="y")
            finish_layer(b1s[:], y1)

            # ================ layer 2 ================
            rec2 = recp.tile([128, tb, trw], f16, tag="rec")
            build_publish(make_y_lhsT(y1), W2s[:], miscS[:, 16:32], rec2,
                          bounce2, table2)
            edge_phase(table2, rec2)
            y2 = recp.tile([128, tb, hc], f16, tag="y")
            finish_layer(b2s[:], y2)

            # ================ output projection ================
            outt = recp.tile([128, tb, ncls], f16, tag="outt")
            y_lhsT2 = make_y_lhsT(y2)
            for t in range(tb):
                yTs = y_lhsT2(t)
                o_p = psH.tile([128, hc + 16], f32, tag="psH")
                nc.tensor.matmul(out=o_p[:, 0:ncls], lhsT=yTs,
                                 rhs=miscS[:, 32:32 + ncls],
                                 start=True, stop=True)
                nc.vector.tensor_tensor(
                    out=outt[:, t, :], in0=o_p[:, 0:ncls], in1=bouts[:],
                    op=Alu.add,
                )
            nc.sync.dma_start(
                out[:].rearrange("(p t) w -> p t w", p=128), outt[:]
            )

    nc.compile()
    return nc


# ---------------------------------------------------------------- entry point

_CACHE = {}


def prepare(inputs, c):
    """inputs dict -> (in_maps, sched, perm)."""
    x = np.asarray(inputs["x"], np.float32)
    edge_index = np.asarray(inputs["edge_index"])
    edge_maps, sched, perm = host_prep(x, edge_index, c)
    packs = host_pack(
        x, edge_maps, sched, perm, inputs["W1"], inputs["a_src1"],
        inputs["a_dst1"], inputs["b1"], inputs["W2"], inputs["a_src2"],
        inputs["a_dst2"], inputs["b2"], inputs["Wout"], inputs["bout"], c)
    in_maps = [dict(pack=p) for p in packs]
    return in_maps, sched, perm


def kernel(x, edge_index, W1, a_src1, a_dst1, b1, W2, a_src2, a_dst2, b2,
           Wout, bout):
    from concourse.bass_utils import run_bass_kernel_spmd

    c = derive(full_cfg())
    in_maps, sched, perm = prepare(dict(
        x=x, edge_index=edge_index, W1=W1, a_src1=a_src1, a_dst1=a_dst1,
        b1=b1, W2=W2, a_src2=a_src2, a_dst2=a_dst2, b2=b2, Wout=Wout,
        bout=bout), c)
    key = ("full", sched["tpw"].tobytes())
    if key not in _CACHE:
        _CACHE[key] = build_nc(c, sched)
    nc = _CACHE[key]
    res = run_bass_kernel_spmd(nc, in_maps, list(range(c["cores"])))
    return host_post(res.results, perm, c)
